# revision 1
# baseline (speedup 1.0000x reference)
"""Trainium2 Bass kernel for nn_CIN_81544249082266 (CIN / xDeepFM cross network).

Pure data parallel over 8 NeuronCores: each core processes 1024 of the 8192
batch rows; filters and output weights are replicated. No cross-device
communication (the host concatenates the per-core [1024] score vectors).

Math (per sample b, embedding dim d in [0,16), fields F0=39):
  layer k: z[(i,j), (b,d)] = x0[i,(b,d)] * h_k[j,(b,d)];  curr = relu(F_k^T z)
  h_{k+1} = curr rows [0:64), direct_k = remaining rows
  score[b] = sum_{m,d} direct[m,(b,d)] * (1 + w_nn[m]) + b_nn

On-chip layout: everything lives transposed ("rows" r=(b*16+d) on the free
axis), so each layer's PSUM output [128 l-partitions, r] is directly the next
layer's h operand — no transposes anywhere. The replicated-x operand A
(row i broadcast across 64 partitions) is materialized by broadcast DMA;
the tiled-h operand is materialized by writing the relu output twice.
"""

import numpy as np
from contextlib import ExitStack

import concourse.bass as bass
import concourse.tile as tile
from concourse import bacc, mybir
from concourse.bass_utils import run_bass_kernel_spmd

F0 = 39
D = 16
B = 8192
NCORES = 8
BC = B // NCORES            # 1024 samples per core
N = BC * D                  # 16384 r-columns per core
CH = 1024                   # chunk of r processed per inner iteration
NCHUNK = N // CH            # 16
NBLK = 20                   # ceil(40*64/128) c-blocks per layer (i padded to 40)
NBLK_L0 = 19                # block 19 (i=38,39) is all-zero after the triu mask
FP16 = mybir.dt.float16
FP32 = mybir.dt.float32

_BUILT = None


def _build_program():
    """Build + compile the 8-core SPMD Bass program once per process."""
    nc = bacc.Bacc(
        "TRN2",
        target_bir_lowering=False,
        debug=False,
        num_devices=NCORES,
    )

    x2_d = nc.dram_tensor("x2", [64, N], FP16, kind="ExternalInput").ap()
    f_d = [
        nc.dram_tensor(f"f{k}", [128, NBLK * 128], FP16, kind="ExternalInput").ap()
        for k in range(3)
    ]
    wv_d = nc.dram_tensor("wv", [128, 3], FP16, kind="ExternalInput").ap()
    bias_d = nc.dram_tensor("bias", [1, 1], FP32, kind="ExternalInput").ap()
    out_d = nc.dram_tensor("out", [1, BC], FP32, kind="ExternalOutput").ap()

    relu = mybir.ActivationFunctionType.Relu
    ndma = [0]

    with tile.TileContext(nc) as tc, ExitStack() as ctx:
        const = ctx.enter_context(tc.tile_pool(name="const", bufs=1))
        pool_a = ctx.enter_context(tc.tile_pool(name="a", bufs=32))
        pool_x = ctx.enter_context(tc.tile_pool(name="x", bufs=5))
        pool_z = ctx.enter_context(tc.tile_pool(name="z", bufs=5))
        pool_h = ctx.enter_context(tc.tile_pool(name="h", bufs=3))
        pool_dt = ctx.enter_context(tc.tile_pool(name="dt", bufs=3))
        pool_r2 = ctx.enter_context(tc.tile_pool(name="r2", bufs=3))
        ps_curr = ctx.enter_context(tc.tile_pool(name="pcur", bufs=3, space="PSUM"))
        ps_s = ctx.enter_context(tc.tile_pool(name="ps", bufs=2, space="PSUM"))

        # --- resident constants ---
        fsb = []
        for k in range(3):
            f = const.tile([128, NBLK * 128], FP16, tag=f"f{k}")
            nc.gpsimd.dma_start(f[:, : NBLK * 64], f_d[k][:, : NBLK * 64])
            nc.gpsimd.dma_start(f[:, NBLK * 64 :], f_d[k][:, NBLK * 64 :])
            fsb.append(f)
        wv = const.tile([128, 3], FP16)
        nc.sync.dma_start(wv[:], wv_d[:])
        bias = const.tile([1, 1], FP32)
        nc.sync.dma_start(bias[:], bias_d[:])
        scores = const.tile([1, BC], FP32)

        def load_chunk(c):
            """A-broadcast DMAs + x-chunk for chunk c, straight from HBM.
            The x-chunk and low-k tiles are needed first, so they go on the
            gpsimd SW DGE (sprays across all 16 SDMA engines); the sync/
            scalar HW queues (one shared slow 2-engine SDMA pair) only get
            the last two k's of each chunk."""
            sl = slice(c * CH, (c + 1) * CH)
            xc = pool_x.tile([128, CH], FP16, tag="xc", name=f"xc_{c}")
            nc.sync.dma_start(xc[0:64, :], x2_d[:, sl])
            nc.scalar.dma_start(xc[64:128, :], x2_d[:, sl])
            a_tiles = []
            for j in range(NBLK // 2):
                a = pool_a.tile([128, 2 * CH], FP16, tag="a", name=f"a_{c}_{j}")
                for s in range(2):
                    k = 2 * j + s
                    src = x2_d[2 * k : 2 * k + 2, None, sl].to_broadcast([2, 64, CH])
                    nc.gpsimd.dma_start(a[:, s * CH : (s + 1) * CH], src)
                a_tiles.append(a)
            return a_tiles, xc

        def layer_pass(c, layer, a_tiles, b_op, fw):
            """One 1024-wide k-sweep: z = a*b on DVE feeding accumulating
            matmuls; returns the PSUM tile."""
            nblk = NBLK_L0 if layer == 0 else NBLK
            cur = ps_curr.tile([128, CH], FP32, tag="cur", name=f"cur_{c}_{layer}")
            for j in range((nblk + 1) // 2):
                ns = min(2, nblk - 2 * j)      # 1 for L0's last (k=18) op
                z = pool_z.tile([128, 2 * CH], FP16, tag="z", name=f"z_{c}_{layer}_{j}")
                nc.vector.tensor_tensor(
                    out=z[:, : ns * CH].rearrange("p (s c) -> p s c", s=ns),
                    in0=a_tiles[j][:, : ns * CH].rearrange("p (s c) -> p s c", s=ns),
                    in1=b_op[:, None, :].to_broadcast([128, ns, CH]),
                    op=mybir.AluOpType.mult,
                )
                for s in range(ns):
                    k = 2 * j + s
                    for sgn in range(2):
                        ssl = slice(s * CH + sgn * 512, s * CH + (sgn + 1) * 512)
                        nc.tensor.matmul(
                            cur[:, sgn * 512 : (sgn + 1) * 512],
                            lhsT=fw[:, k * 128 : (k + 1) * 128],
                            rhs=z[:, ssl],
                            start=(k == 0),
                            stop=(k == nblk - 1),
                        )
            return cur

        def score_mms(sab, layer, rhs_t, rhs_k):
            for sgn in range(2):
                ssl = slice(sgn * 512, (sgn + 1) * 512)
                nc.tensor.matmul(
                    sab[32 * sgn : 32 * sgn + 1, :],
                    lhsT=wv[0:rhs_k, layer : layer + 1],
                    rhs=rhs_t[0:rhs_k, ssl],
                    start=(layer == 0), stop=(layer == 2),
                    tile_position=(0, 32 * sgn),
                )

        # Software-pipelined schedule: L0 runs one chunk ahead so the PE
        # always has an independent pass to stream while the previous pass
        # drains through ACT (relu) into the next layer's operand.
        def do_l0(c, a_tiles, xc):
            cur = layer_pass(c, 0, a_tiles, xc, fsb[0])
            h_t = pool_h.tile([128, CH], FP16, tag="h", name=f"h_{c}")
            d_t = pool_dt.tile([64, CH], FP16, tag="d", name=f"d_{c}")
            nc.scalar.activation(h_t[0:64, :], cur[0:64, :], relu)
            nc.scalar.activation(h_t[64:128, :], cur[0:64, :], relu)
            nc.scalar.activation(d_t[:], cur[64:128, :], relu)
            sab = ps_s.tile([33, 512], FP32, tag="sab", name=f"sab_{c}")
            score_mms(sab, 0, d_t, 64)
            return h_t, sab

        chunks = {}
        for cc in range(3):
            chunks[cc] = load_chunk(cc)
        h_t, sab = do_l0(0, chunks[0][0], chunks[0][1])
        state = (h_t, sab)

        def emit_reduces(t, sab_t):
            # sum over d (innermost 16): each half-score row -> 32 b-scores
            for sgn in range(2):
                off = t * (CH // D) + sgn * 32
                nc.vector.tensor_reduce(
                    out=scores[0:1, off : off + 32],
                    in_=sab_t[32 * sgn : 32 * sgn + 1, :].rearrange(
                        "p (g x) -> p g x", x=D
                    ),
                    axis=mybir.AxisListType.X,
                    op=mybir.AluOpType.add,
                )

        pending_reduce = None
        for t in range(NCHUNK):
            if t + 3 < NCHUNK:
                chunks[t + 3] = load_chunk(t + 3)
            a_tiles, _ = chunks[t]
            h1, sab = state
            # L1(t)
            cur1 = layer_pass(t, 1, a_tiles, h1, fsb[1])
            # previous chunk's L2 score MMs + reduce, emitted here (r2 and
            # sab have been ready since mid-previous iteration) so neither
            # the PE nor the DVE ever stalls on them
            if pending_reduce is not None:
                pt, psab, pr2 = pending_reduce
                score_mms(psab, 2, pr2, 128)
                emit_reduces(pt, psab)
            h2 = pool_h.tile([128, CH], FP16, tag="h", name=f"h2_{t}")
            d1 = pool_dt.tile([64, CH], FP16, tag="d", name=f"d1_{t}")
            nc.scalar.activation(h2[0:64, :], cur1[0:64, :], relu)
            nc.scalar.activation(h2[64:128, :], cur1[0:64, :], relu)
            nc.scalar.activation(d1[:], cur1[64:128, :], relu)
            # L0(t+1) streams while L1(t) drains through ACT
            if t + 1 < NCHUNK:
                state = do_l0(t + 1, chunks[t + 1][0], chunks[t + 1][1])
            score_mms(sab, 1, d1, 64)
            # L2(t)
            cur2 = layer_pass(t, 2, a_tiles, h2, fsb[2])
            r2 = pool_r2.tile([128, CH], FP16, tag="r2", name=f"r2_{t}")
            nc.scalar.activation(r2[:], cur2[:], relu)
            del chunks[t]
            pending_reduce = (t, sab, r2)

        pt, psab, pr2 = pending_reduce
        score_mms(psab, 2, pr2, 128)
        emit_reduces(pt, psab)
        nc.vector.tensor_scalar_add(scores[:], scores[:], bias[0:1, 0:1])
        nc.sync.dma_start(out_d[:], scores[:])

    nc.compile()
    return nc


def _prep_inputs(nn_input, f0, f1, f2, w_nn, b_nn):
    """Host-side preprocessing into the kernel's layouts."""
    nn_input = np.asarray(nn_input, dtype=np.float32)
    f0 = np.asarray(f0, dtype=np.float32)
    f1 = np.asarray(f1, dtype=np.float32)
    f2 = np.asarray(f2, dtype=np.float32)
    w_nn = np.asarray(w_nn, dtype=np.float32).reshape(-1)
    b_nn = np.asarray(b_nn, dtype=np.float32).reshape(-1)

    # filters -> [40*64, 128] (i-major, j in [0,64)), then lhsT blocks
    def pack(fp):  # fp: [2560, 128] -> [128, NBLK*128]
        blocks = fp.reshape(NBLK, 128, 128)
        return np.ascontiguousarray(
            blocks.transpose(1, 0, 2).reshape(128, NBLK * 128)
        ).astype(np.float16)

    f0p = np.zeros((40, 64, 128), np.float32)
    f0r = f0.reshape(F0, F0, 128)
    iu, ju = np.triu_indices(F0, k=1)
    f0p[iu, ju] = 2.0 * f0r[iu, ju]
    f0p = pack(f0p.reshape(2560, 128))

    def padf(f):  # [39*64, 128] -> [2560, 128]
        out = np.zeros((2560, 128), np.float32)
        out[: F0 * 64] = f
        return pack(out)

    f1p, f2p = padf(f1), padf(f2)

    wv = np.zeros((128, 3), np.float32)
    wv[0:64, 0] = 1.0 + w_nn[0:64]    # layer-0 direct weights (res + w_nn)
    wv[0:64, 1] = 1.0 + w_nn[64:128]  # layer-1 direct weights
    wv[:, 2] = 1.0 + w_nn[128:256]    # layer-2 direct weights
    wv = wv.astype(np.float16)
    bias = b_nn.reshape(1, 1).astype(np.float32)

    # x2 per core: [64, N] fp16 with rows 0..38 = x^T, rest zero
    x0 = nn_input.reshape(B, F0, D)
    in_maps = []
    for cidx in range(NCORES):
        xc = x0[cidx * BC : (cidx + 1) * BC]            # [BC, 39, 16]
        xt = xc.transpose(1, 0, 2).reshape(F0, N)        # [39, (b,d)]
        x2h = np.zeros((64, N), np.float16)
        x2h[:F0] = xt.astype(np.float16)
        in_maps.append(
            {"x2": x2h, "f0": f0p, "f1": f1p, "f2": f2p, "wv": wv, "bias": bias}
        )
    return in_maps


def _run(inputs, trace=False, trace_kwargs=None):
    global _BUILT
    if _BUILT is None:
        _BUILT = _build_program()
    nc = _BUILT
    in_maps = _prep_inputs(**inputs)
    res = run_bass_kernel_spmd(
        nc,
        in_maps,
        core_ids=list(range(NCORES)),
        trace=trace,
        **(trace_kwargs or {}),
    )
    out = np.concatenate(
        [res.results[c]["out"].reshape(BC) for c in range(NCORES)]
    )
    return out.reshape(B, 1).astype(np.float32), res


def kernel(**inputs):
    out, _ = _run(inputs)
    return out



# revision 33
# speedup vs baseline: 1.1002x; 1.1002x over previous
"""Trainium2 Bass kernel for nn_CIN_81544249082266 (CIN / xDeepFM cross network).

Pure data parallel over 8 NeuronCores: each core processes 1024 of the 8192
batch rows; filters and output weights are replicated.

Math (per sample b, embedding dim d in [0,16), fields F0=39):
  layer k: z[(i,j), (b,d)] = x0[i,(b,d)] * h_k[j,(b,d)];  curr = relu(F_k^T z)
  h_{k+1} = curr rows [0:64), direct_k = remaining rows
  score[b] = sum_{m,d} direct[m,(b,d)] * (1 + w_nn[m]) + b_nn

Layer split (vs. the all-TT baseline, which was DVE-bound on z = a*h):

* L1/L2 (TT path): z = a*h on DVE (fp16, 2x_1p mode, 2-block granule),
  a = x_i broadcast-DMA tiles (HBM source, gpsimd SWDGE spray); fp16
  512-col matmuls emitted TT-first so the PSUM accumulation group never
  stalls the PE waiting on slower producers.
* L0 (square path): uses x_i*x_j = ((x_i+x_j)^2 - x_i^2 - x_j^2)/2 over
  the strictly-upper triangle (741 pairs packed into 6 c-blocks instead
  of 19). Pair sums s are computed ON THE PE as fp8 DoubleRow matmuls
  with a 0/1 selection matrix A (ktile0 = A over x8, ktile1 = A over
  x8res -- the residual ktile makes x exact to ~0.2%). ACT squares s
  (PSUM fp32 -> SBUF fp8); the main contraction runs as fp8 DoubleRow
  matmuls (2 c-blocks per instruction). The -x^2 correction runs as two
  plain-fp8 matmuls (hi + residual) over vsq = q8(x16^2), with weights
  derived from the QUANTIZED mains so fp8 weight noise cancels against
  the s^2 mean instead of leaking into curr.
  This kills 19 of 59 DVE z-blocks and 1/3 of the broadcast-DMA bytes
  for ~2 us/chunk of cheap PE/ACT work.

Measured on HW: ~620-640 us (traced), rel err ~9.1e-3 vs fp32 reference
(gate 2e-2). Bottleneck: broadcast a-tile DMA (~575 us across the 16
SDMA engines at the observed ~160 GB/s effective broadcast rate); DVE
~375 us, PE ~465 us, ACT ~270 us. The square path for L1/L2 (h-based)
was measured NET-SLOWER at every TT/SQ split tried (vt/h8 chain adds
ACT+latency exceeding the DMA/DVE savings), hence SQ_KS = [].

All fp8 weights are scaled by SC=64 (keeps N(0,0.02) filter values out of
the e4m3 subnormal range); relu's scale argument de-scales by 1/SC.
"""

import numpy as np
from contextlib import ExitStack

import concourse.bass as bass
import concourse.tile as tile
from concourse import bacc, mybir
from concourse.bass_utils import run_bass_kernel_spmd

F0 = 39
D = 16
B = 8192
NCORES = 8
BC = B // NCORES            # 1024 samples per core
N = BC * D                  # 16384 r-columns per core
CH = 1024                   # chunk of r processed per inner iteration
NCHUNK = N // CH            # 16
SC = 64.0                   # fp8 weight scale

# L1/L2 k-blocks (k covers i = 2k, 2k+1, j in [0,64)): TT path vs square path
TT_KS = list(range(0, 20))
SQ_KS = []
AGRP = 4                    # TT ks per a-tile / z-tile group
GPS_GROUPS = set()              # (layer, group) z-TT ops run on GPSIMD
NL0 = 6                     # L0 packed pair-blocks (741 pairs -> 6*128)

FP16 = mybir.dt.float16
FP32 = mybir.dt.float32
FP8 = mybir.dt.float8e4
DR = mybir.MatmulPerfMode.DoubleRow

_BUILT = None


def _build_program():
    nc = bacc.Bacc(
        "TRN2",
        target_bir_lowering=False,
        debug=False,
        num_devices=NCORES,
    )

    n16 = len(TT_KS)
    nsq = len(SQ_KS)
    ndr12 = nsq // 2            # DoubleRow main pairs per layer (L1/L2)
    ndr0 = NL0 // 2             # L0 DR main pairs

    x16_d = nc.dram_tensor("x16", [64, N], FP16, kind="ExternalInput").ap()
    # rows 0-38: x8 | x8res; rows 39-127: zeros (DMA'd into vt so every
    # weight-is-zero region holds real zeros, never NaN fp8 garbage).
    # v layout: x at rows 0-38 (pad to 64), h at rows 64-127 (32-aligned so
    # the ACT relu can write h8 from cur[0:64] -> vt[64:128]).
    xv8_d = nc.dram_tensor("xv8", [128, 2 * N], FP8, kind="ExternalInput").ap()
    # fp16 TT-path weights: [L1 blocks | L2 blocks]
    f16_d = nc.dram_tensor("f16", [128, 2 * n16 * 128], FP16,
                           kind="ExternalInput").ap()
    # fp8 DR main weights: [L0 pairs | L1 pairs | L2 pairs], 256 cols per pair
    f8_d = nc.dram_tensor("f8", [128, (ndr0 + 2 * ndr12) * 256], FP8,
                          kind="ExternalInput").ap()
    a0_d = nc.dram_tensor("a0", [64, NL0 * 256], FP8, kind="ExternalInput").ap()
    a1_d = (nc.dram_tensor("a1", [128, nsq * 256], FP8,
                           kind="ExternalInput").ap() if nsq else None)
    # corrections: x^2 rows 0-38, h^2 rows 64-127; per layer (hi|res) pairs
    # computed from the QUANTIZED main weights so the fp8 weight noise cancels
    # against the s^2 mean instead of leaking into curr.
    corr_d = nc.dram_tensor("corr", [128, 6 * 128], FP8, kind="ExternalInput").ap()
    wv_d = nc.dram_tensor("wv", [128, 3], FP16, kind="ExternalInput").ap()
    bias_d = nc.dram_tensor("bias", [1, 1], FP32, kind="ExternalInput").ap()
    out_d = nc.dram_tensor("out", [1, BC], FP32, kind="ExternalOutput").ap()

    relu = mybir.ActivationFunctionType.Relu
    square = mybir.ActivationFunctionType.Square

    with tile.TileContext(nc) as tc, ExitStack() as ctx:
        const = ctx.enter_context(tc.tile_pool(name="const", bufs=1))
        pool_a = ctx.enter_context(tc.tile_pool(name="a", bufs=8))
        pool_x = ctx.enter_context(tc.tile_pool(name="x", bufs=3))
        pool_v = ctx.enter_context(tc.tile_pool(name="v", bufs=3))
        pool_z = ctx.enter_context(tc.tile_pool(name="z", bufs=5))
        pool_z8 = ctx.enter_context(tc.tile_pool(name="z8", bufs=5))
        pool_vsq = ctx.enter_context(tc.tile_pool(name="vsq", bufs=2))
        pool_h = ctx.enter_context(tc.tile_pool(name="h", bufs=3))
        pool_dt = ctx.enter_context(tc.tile_pool(name="dt", bufs=3))
        pool_r2 = ctx.enter_context(tc.tile_pool(name="r2", bufs=3))
        ps_cur = ctx.enter_context(tc.tile_pool(name="pcur", bufs=2, space="PSUM"))
        ps_s = ctx.enter_context(tc.tile_pool(name="ps_s", bufs=2, space="PSUM"))
        ps_sab = ctx.enter_context(tc.tile_pool(name="psab", bufs=2, space="PSUM"))

        # --- resident constants ---
        f16sb = const.tile([128, 2 * n16 * 128], FP16)
        nc.gpsimd.dma_start(f16sb[:, : n16 * 128], f16_d[:, : n16 * 128])
        nc.gpsimd.dma_start(f16sb[:, n16 * 128:], f16_d[:, n16 * 128:])
        f8sb = const.tile([128, (ndr0 + 2 * ndr12) * 256], FP8)
        nc.sync.dma_start(f8sb[:], f8_d[:])
        a0sb = const.tile([64, NL0 * 256], FP8)
        nc.sync.dma_start(a0sb[:], a0_d[:])
        a1sb = None
        if nsq:
            a1sb = const.tile([128, nsq * 256], FP8)
            nc.sync.dma_start(a1sb[:], a1_d[:])
        corrsb = const.tile([128, 6 * 128], FP8)
        nc.sync.dma_start(corrsb[:], corr_d[:])
        wv = const.tile([128, 3], FP16)
        nc.sync.dma_start(wv[:], wv_d[:])
        bias = const.tile([1, 1], FP32)
        nc.sync.dma_start(bias[:], bias_d[:])
        scores = const.tile([1, BC], FP32)

        def t2(ap):
            return ap.rearrange("p (t m) -> p t m", t=2)

        def load_chunk(c):
            sl = slice(c * CH, (c + 1) * CH)
            xc = pool_x.tile([64, CH], FP16, tag="xc", name=f"xc_{c}")
            nc.sync.dma_start(xc[:], x16_d[:, sl])
            vts = []
            nvt = 3 if nsq else 1
            for li in range(nvt):
                vt = pool_v.tile([128, 2 * CH], FP8, tag="vt", name=f"vt_{c}_{li}")
                nc.scalar.dma_start(vt[0:64, 0:CH], xv8_d[0:64, sl])
                rows = 128 if li > 0 else 64
                nc.scalar.dma_start(
                    vt[0:rows, CH: 2 * CH],
                    xv8_d[0:rows, N + c * CH: N + (c + 1) * CH],
                )
                vts.append(vt)
            while len(vts) < 3:
                vts.append(None)
            a_tiles = []
            for g in range(n16 // AGRP):
                a = pool_a.tile([128, AGRP * CH], FP16, tag="a", name=f"a_{c}_{g}")
                for s in range(AGRP):
                    k = TT_KS[AGRP * g + s]
                    src = x16_d[2 * k: 2 * k + 2, None, sl].to_broadcast([2, 64, CH])
                    nc.gpsimd.dma_start(a[:, s * CH: (s + 1) * CH], src)
                a_tiles.append(a)
            return a_tiles, xc, vts

        def sq_blocks(c, layer, vt, nblk, asb, acol0, kmax):
            """Square path: sgen DR matmuls -> squares -> fp8 z' tiles.
            Returns list of [128, 2*CH] fp8 tiles (2 c-blocks each)."""
            zts = []
            for p in range(nblk // 2):
                z8t = pool_z8.tile([128, 2 * CH], FP8, tag="z8",
                                   name=f"z8_{c}_{layer}_{p}")
                for s in range(2):          # the 2 c-blocks of this pair
                    blk = 2 * p + s
                    lhs = t2(asb[0:kmax, (acol0 + blk) * 256:
                                 (acol0 + blk + 1) * 256])
                    for half in range(2):
                        st = ps_s.tile([128, 512], FP32, tag="s",
                                       name=f"s_{c}_{layer}_{blk}_{half}")
                        rhs = vt[0:kmax].rearrange(
                            "p (t ch) -> p t ch", t=2
                        )[:, :, half * 512: (half + 1) * 512]
                        nc.tensor.matmul(st[:], lhsT=lhs, rhs=rhs,
                                         perf_mode=DR, skip_group_check=True)
                        dst = z8t[:, s * CH + half * 512: s * CH + (half + 1) * 512]
                        nc.scalar.activation(dst, st[:], square)
                zts.append(z8t)
            return zts

        def mains(c, layer, zts8, z16s, fcol0, n16blk, f16col0, corr_col, kmax,
                  with_corr=True):
            """Accumulate cur = sum of DR fp8 mains + fp16 TT mains (+ corr)."""
            cur = ps_cur.tile([128, CH], FP32, tag="cur", name=f"cur_{c}_{layer}")
            # TT mains first: their z is ready long before the sgen->square
            # chain finishes, so the PSUM group starts without stalling the PE
            first = True
            nz = len(z16s)
            zb = AGRP
            for gi, zt in enumerate(z16s):
                for s in range(zb):
                    kk = gi * zb + s
                    for half in range(2):
                        nc.tensor.matmul(
                            cur[:, half * 512: (half + 1) * 512],
                            lhsT=f16sb[:, (f16col0 + kk) * 128:
                                       (f16col0 + kk + 1) * 128],
                            rhs=zt[:, s * CH + half * 512:
                                   s * CH + (half + 1) * 512],
                            start=first, stop=False, skip_group_check=True)
                    first = False
            np8 = len(zts8)
            for p, z8t in enumerate(zts8):
                lhs = t2(f8sb[:, (fcol0 + p) * 256: (fcol0 + p + 1) * 256])
                last = (not with_corr) and p == np8 - 1
                for half in range(2):
                    rhs = z8t.rearrange("p (t ch) -> p t ch", t=2)[
                        :, :, half * 512: (half + 1) * 512]
                    nc.tensor.matmul(
                        cur[:, half * 512: (half + 1) * 512],
                        lhsT=lhs, rhs=rhs, perf_mode=DR,
                        start=first, stop=last, skip_group_check=True)
                first = False
            return cur

        def corr_mm(cur, vsq, corr_col, kmax):
            for half in range(2):
                for rr in range(2):
                    nc.tensor.matmul(
                        cur[:, half * 512: (half + 1) * 512],
                        lhsT=corrsb[0:kmax, (2 * corr_col + rr) * 128:
                                    (2 * corr_col + rr + 1) * 128],
                        rhs=vsq[0:kmax, half * 512: (half + 1) * 512],
                        start=False, stop=(rr == 1), skip_group_check=True)

        def score_mms(sab, layer, rhs_t, rhs_k):
            for sgn in range(2):
                ssl = slice(sgn * 512, (sgn + 1) * 512)
                nc.tensor.matmul(
                    sab[32 * sgn: 32 * sgn + 1, :],
                    lhsT=wv[0:rhs_k, layer: layer + 1],
                    rhs=rhs_t[0:rhs_k, ssl],
                    start=(layer == 0), stop=(layer == 2),
                    tile_position=(0, 32 * sgn),
                )

        def emit_reduces(t, sab_t):
            for sgn in range(2):
                off = t * (CH // D) + sgn * 32
                nc.vector.tensor_reduce(
                    out=scores[0:1, off: off + 32],
                    in_=sab_t[32 * sgn: 32 * sgn + 1, :].rearrange(
                        "p (g x) -> p g x", x=D
                    ),
                    axis=mybir.AxisListType.X,
                    op=mybir.AluOpType.add,
                )

        ISC = 1.0 / SC

        def do_l0(c, xc, vt0):
            """L0 square path; produces bt1 (h fp16 x2), vsq (x^2 part),
            sab, and writes h8 into vt1."""
            vsq = pool_vsq.tile([128, CH], FP8, tag="vsq", name=f"vsq_{c}")
            nc.scalar.activation(vsq[0:64, :], xc[:], square)
            zts = sq_blocks(c, 0, vt0, NL0, a0sb, 0, 64)
            cur = mains(c, 0, zts, [], 0, 0, 0, 0, 64)
            corr_mm(cur, vsq, 0, 64)
            return cur, vsq

        def relu_boundary(c, layer, cur, vt_next):
            """relu(cur/SC): bt (h fp16 double), dt (direct fp16), h8->vt."""
            bt = pool_h.tile([128, CH], FP16, tag="h", name=f"bt_{c}_{layer}")
            dt = pool_dt.tile([64, CH], FP16, tag="d", name=f"dt_{c}_{layer}")
            nc.scalar.activation(bt[0:64, :], cur[0:64, :], relu, scale=ISC)
            nc.scalar.activation(bt[64:128, :], cur[0:64, :], relu, scale=ISC)
            nc.scalar.activation(dt[:], cur[64:128, :], relu, scale=ISC)
            if vt_next is not None:
                nc.scalar.activation(vt_next[64:128, 0:CH], cur[0:64, :], relu,
                                     scale=ISC)
            return bt, dt

        def do_l12(c, layer, bt, vt, vsq, a_tiles):
            if nsq:
                # h^2 for this layer's correction (consistent with h8 in s)
                nc.scalar.activation(vsq[64:128, :], vt[64:128, 0:CH], square)
            zts8 = sq_blocks(c, layer, vt, nsq, a1sb, 0, 128) if nsq else []
            z16s = []
            hg_n = AGRP
            for g, a in enumerate(a_tiles):
                for hg in range(1):
                    zt = pool_z.tile([128, hg_n * CH], FP16, tag="z",
                                     name=f"z_{c}_{layer}_{g}_{hg}")
                    eng = nc.gpsimd if (layer, 2 * g + hg) in GPS_GROUPS \
                        else nc.vector
                    eng.tensor_tensor(
                        out=zt[:].rearrange("p (s ch) -> p s ch", s=hg_n),
                        in0=a[:, hg * hg_n * CH: (hg + 1) * hg_n * CH].rearrange(
                            "p (s ch) -> p s ch", s=hg_n),
                        in1=bt[:, None, :].to_broadcast([128, hg_n, CH]),
                        op=mybir.AluOpType.mult,
                    )
                    z16s.append(zt)
            fcol0 = ndr0 + (layer - 1) * ndr12
            f16col0 = (layer - 1) * n16
            cur = mains(c, layer, zts8, z16s, fcol0, n16, f16col0, layer, 128,
                        with_corr=bool(nsq))
            if nsq:
                corr_mm(cur, vsq, layer, 128)
            return cur

        # --- pipelined schedule (baseline skeleton) ---
        chunks = {}
        for cc in range(2):
            chunks[cc] = load_chunk(cc)

        def l0_stage(c):
            a_tiles, xc, vts = chunks[c]
            cur0, vsq = do_l0(c, xc, vts[0])
            bt1, d0 = relu_boundary(c, 0, cur0, vts[1])
            sab = ps_sab.tile([33, 512], FP32, tag="sab", name=f"sab_{c}")
            score_mms(sab, 0, d0, 64)
            return bt1, vsq, sab

        state = l0_stage(0)
        pending_reduce = None
        for t in range(NCHUNK):
            if t + 2 < NCHUNK:
                chunks[t + 2] = load_chunk(t + 2)
            a_tiles, xc, vts = chunks[t]
            bt1, vsq, sab = state
            cur1 = do_l12(t, 1, bt1, vts[1], vsq, a_tiles)
            if pending_reduce is not None:
                pt, psab, pr2 = pending_reduce
                score_mms(psab, 2, pr2, 128)
                emit_reduces(pt, psab)
            bt2, d1 = relu_boundary(t, 1, cur1, vts[2])
            if t + 1 < NCHUNK:
                state = l0_stage(t + 1)
            score_mms(sab, 1, d1, 64)
            cur2 = do_l12(t, 2, bt2, vts[2], vsq, a_tiles)
            r2 = pool_r2.tile([128, CH], FP16, tag="r2", name=f"r2_{t}")
            nc.scalar.activation(r2[:], cur2[:], relu, scale=ISC)
            del chunks[t]
            pending_reduce = (t, sab, r2)

        pt, psab, pr2 = pending_reduce
        score_mms(psab, 2, pr2, 128)
        emit_reduces(pt, psab)
        nc.vector.tensor_scalar_add(scores[:], scores[:], bias[0:1, 0:1])
        nc.sync.dma_start(out_d[:], scores[:])

    nc.compile()
    return nc


def _prep_inputs(nn_input, f0, f1, f2, w_nn, b_nn):
    import ml_dtypes
    F8NP = ml_dtypes.float8_e4m3

    nn_input = np.asarray(nn_input, dtype=np.float32)
    f0 = np.asarray(f0, dtype=np.float32).reshape(F0, F0, 128)
    f1 = np.asarray(f1, dtype=np.float32).reshape(F0, 64, 128)
    f2 = np.asarray(f2, dtype=np.float32).reshape(F0, 64, 128)
    w_nn = np.asarray(w_nn, dtype=np.float32).reshape(-1)
    b_nn = np.asarray(b_nn, dtype=np.float32).reshape(-1)

    n16 = len(TT_KS)
    nsq = len(SQ_KS)
    ndr12 = nsq // 2
    ndr0 = NL0 // 2
    iu, ju = np.triu_indices(F0, k=1)          # 741 pairs, i-major
    

    def q8(a):
        return np.asarray(a, dtype=F8NP)

    # --- fp16 TT weights (scaled by SC): blocks k -> lhsT [128, 128] ---
    def f16_pack(f):
        # f: [39, 64, 128] -> padded [40,64,128]; block k rows = (i,j) pairs
        fp = np.zeros((40, 64, 128), np.float32)
        fp[:F0] = f * SC
        blocks = fp.reshape(20, 128, 128)
        out = np.zeros((128, n16 * 128), np.float32)
        for idx, k in enumerate(TT_KS):
            out[:, idx * 128: (idx + 1) * 128] = blocks[k]
        return out
    f16p = np.concatenate([f16_pack(f1), f16_pack(f2)], axis=1).astype(np.float16)

    # --- L0 packed pairs ---
    npair_pad = NL0 * 128
    w0 = np.zeros((npair_pad, 128), np.float32)
    w0[: len(iu)] = q8(f0[iu, ju] * SC).astype(np.float32)   # 0.5 * 2 * f0
    # A0: [64, NL0, 2, 128]; both ktiles select x rows (x8 then x8res)
    a0 = np.zeros((64, NL0, 2, 128), np.float32)
    for m in range(len(iu)):
        blk, pos = divmod(m, 128)
        a0[iu[m], blk, :, pos] = 1.0
        a0[ju[m], blk, :, pos] = 1.0
    # L0 correction from the QUANTIZED weights: -(sum of pairs containing i)
    c0 = np.zeros((F0, 128), np.float32)
    np.add.at(c0, iu, w0[: len(iu)])
    np.add.at(c0, ju, w0[: len(iu)])
    c0 = -c0

    # --- L1/L2 square blocks (pre-quantized so corrections match) ---
    def f8_pack(f):
        fp = np.zeros((40, 64, 128), np.float32)
        fp[:F0] = q8(0.5 * f * SC).astype(np.float32)
        blocks = fp.reshape(20, 128, 128)
        out = np.zeros((ndr12, 2, 128, 128), np.float32)
        for idx, k in enumerate(SQ_KS):
            p, s = divmod(idx, 2)
            out[p, s] = blocks[k]
        # lhsT layout [128, p, t, 128]
        return out.transpose(2, 0, 1, 3).reshape(128, ndr12 * 256), fp
    w0l = w0.reshape(NL0, 128, 128)
    w0dr = np.zeros((ndr0, 2, 128, 128), np.float32)
    for blk in range(NL0):
        p, s = divmod(blk, 2)
        w0dr[p, s] = w0l[blk]
    if nsq:
        f1p8, f1q = f8_pack(f1)
        f2p8, f2q = f8_pack(f2)
    else:
        f1p8 = f2p8 = np.zeros((128, 0), np.float32)
        f1q = f2q = np.zeros((40, 64, 128), np.float32)
    f8p = np.concatenate(
        [w0dr.transpose(2, 0, 1, 3).reshape(128, ndr0 * 256), f1p8, f2p8],
        axis=1)
    f8p = q8(f8p)

    # A1: [128, nsq, 2, 128]: col m -> x-row 2k+m//64 (both ktiles),
    # h-row 64+m%64 (ktile0 only)
    a1 = np.zeros((128, nsq, 2, 128), np.float32)
    for idx, k in enumerate(SQ_KS):
        for m in range(128):
            i = 2 * k + m // 64
            j = m % 64
            if i < F0:
                a1[i, idx, :, m] = 1.0
                a1[64 + j, idx, 0, m] = 1.0
    a0p = q8(a0.reshape(64, NL0 * 256))
    a1p = q8(a1.reshape(128, nsq * 256)) if nsq else None

    # corrections for L1/L2 from the QUANTIZED packed weights, restricted
    # to the square-path i rows (the TT path needs no correction).
    sq_i = sorted({2 * k + s for k in SQ_KS for s in range(2)} & set(range(F0)))
    def corr12(fq):
        out = np.zeros((128, 128), np.float32)
        out[sq_i] = -fq[sq_i].sum(axis=1)
        out[64:128] = -fq[sq_i].sum(axis=0)
        return out
    def hi_res(c):
        hi = q8(c).astype(np.float32)
        return hi, c - hi
    corr = np.zeros((128, 6 * 128), np.float32)
    c0f = np.zeros((128, 128), np.float32)
    c0f[0:39] = c0
    for li, cf in enumerate([c0f, corr12(f1q), corr12(f2q)]):
        hi, res = hi_res(cf)
        corr[:, (2 * li) * 128: (2 * li + 1) * 128] = hi
        corr[:, (2 * li + 1) * 128: (2 * li + 2) * 128] = res
    corrp = q8(corr)

    wv = np.zeros((128, 3), np.float32)
    wv[0:64, 0] = 1.0 + w_nn[0:64]
    wv[0:64, 1] = 1.0 + w_nn[64:128]
    wv[:, 2] = 1.0 + w_nn[128:256]
    wv = wv.astype(np.float16)
    biasp = b_nn.reshape(1, 1).astype(np.float32)

    x0 = nn_input.reshape(B, F0, D)
    in_maps = []
    for cidx in range(NCORES):
        xc = x0[cidx * BC: (cidx + 1) * BC]
        xt = xc.transpose(1, 0, 2).reshape(F0, N).astype(np.float32)
        x16 = np.zeros((64, N), np.float16)
        x16[:F0] = xt.astype(np.float16)
        x8 = q8(x16[:F0].astype(np.float32))
        x8res = q8(x16[:F0].astype(np.float32) - x8.astype(np.float32))
        xv8 = np.zeros((128, 2 * N), F8NP)
        xv8[:F0, :N] = x8
        xv8[:F0, N:] = x8res
        im = {
            "x16": x16, "xv8": xv8, "f16": f16p, "f8": f8p,
            "a0": a0p, "corr": corrp, "wv": wv, "bias": biasp,
        }
        if nsq:
            im["a1"] = a1p
        in_maps.append(im)
    return in_maps


def _run(inputs, trace=False, trace_kwargs=None):
    global _BUILT
    if _BUILT is None:
        _BUILT = _build_program()
    nc = _BUILT
    in_maps = _prep_inputs(**inputs)
    res = run_bass_kernel_spmd(
        nc,
        in_maps,
        core_ids=list(range(NCORES)),
        trace=trace,
        **(trace_kwargs or {}),
    )
    out = np.concatenate(
        [res.results[c]["out"].reshape(BC) for c in range(NCORES)]
    )
    return out.reshape(B, 1).astype(np.float32), res


def kernel(**inputs):
    out, _ = _run(inputs)
    return out
